# revision 8
# baseline (speedup 1.0000x reference)
"""BERT self-attention (B=4, S=1024, HID=1024, NH=16, HD=64) on 8 TRN2 NeuronCores.

Sharding: 8 shards = 4 batches x 2 head-halves. Core c handles batch c%4 and
heads [g*8, g*8+8) with g = c//4: q/k/v projections for its 512 feature
columns and full attention for its 8 heads; no collectives.

v4 design — ACT(exp)-paced flat pipeline:
  - scores computed TRANSPOSED per head pair via row-tiled concurrent K=64
    matmuls; exp on ACT (the 73us/core floor engine); v carries a
    0.25-scaled ones column so ctx row 64 is the softmax denominator;
    normalization (divide) and bv happen ON HOST (sum p = 1).
  - one flat 64-step schedule: step t emits scores(t+2)+exp(t+2) (2-step
    lead so ACT never waits on PE filler bursts; scores PSUM ring bufs=2),
    then ~1.1us of work drained from a deadline-sorted queue of half-chains
    (v-proj, fc1-3 q/k proj, DEFERRED ctx chains) so the v/fc1 front lump
    spreads across the whole kernel instead of stalling ACT early.
  - inputs stream over 3 DMA rings in two priority phases (critical
    hsT+fc0, then wv/fc1-3); the fc0 chase consumes hsT chunks in arrival
    order; 9 back-to-back warm-up matmuls hold a full HAM window so
    everything after runs at 2.4 GHz; fc0 bias-adds split across DVE+ACT.
"""
import os
import sys
from contextlib import ExitStack

for _p in ("/root/.axon_site/_ro/trn_rl_repo", "/opt/trn_rl_repo"):
    if os.path.isdir(_p) and _p not in sys.path:
        sys.path.append(_p)

import numpy as np
import concourse.bacc as bacc
import concourse.mybir as mybir
from concourse import tile
from concourse.bass_utils import run_bass_kernel_spmd

B, S, HID, NH, HD = 4, 1024, 1024, 16, 64
NCORES = 8
FSH = 512  # feature columns per core = 8 heads * 64
HC = 8  # hid contraction chunks of 128
JC = 8  # key/seq chunks of 128
SC = 2  # seq chunks of 512 (queries / moving dim)
FC = 4  # feature chunks of 128 (head pairs)
NHL = 8  # local heads per core
VSCALE = 0.25  # pre-scale on v/ones column to keep fp16 ctx~ in range

F32 = mybir.dt.float32
F16 = mybir.dt.float16
EXP = mybir.ActivationFunctionType.Exp
IDENT = mybir.ActivationFunctionType.Identity


def _build_nc():
    nc = bacc.Bacc(None, target_bir_lowering=False, debug=False)

    hsT = nc.declare_dram_parameter("hsT", [128, HC, S], F16, isOutput=False)
    wqP = nc.declare_dram_parameter("wqP", [128, FC, HC, 128], F16, isOutput=False)
    wkP = nc.declare_dram_parameter("wkP", [128, FC, HC, 128], F16, isOutput=False)
    wvT = nc.declare_dram_parameter("wvT", [128, HC, FSH], F16, isOutput=False)
    bqc = nc.declare_dram_parameter("bqc", [128, FC], F32, isOutput=False)
    bkc = nc.declare_dram_parameter("bkc", [128, FC], F32, isOutput=False)
    mb = nc.declare_dram_parameter("mb", [128, JC], F32, isOutput=False)
    out = nc.declare_dram_parameter("out", [NHL, HD + 1, S], F16, isOutput=True)

    with tile.TileContext(nc) as tc, ExitStack() as ctx:
        ctx.enter_context(
            nc.allow_low_precision(reason="fp16 matmuls; fp32 PSUM accumulate")
        )
        const = ctx.enter_context(tc.tile_pool(name="const", bufs=1))

        hsT_sb = const.tile([128, HC, S], F16, tag="hsT")
        wq_sb = const.tile([128, FC, HC, 128], F16, tag="wq")
        wk_sb = const.tile([128, FC, HC, 128], F16, tag="wk")
        wv_sb = const.tile([128, HC, FSH], F16, tag="wv")
        bq_sb = const.tile([128, FC], F32, tag="bq")
        bk_sb = const.tile([128, FC], F32, tag="bk")
        mb_sb = const.tile([128, JC], F32, tag="mb")
        qT_sb = const.tile([128, FC, S], F16, tag="qT")
        kT_sb = const.tile([128, FC, S], F16, tag="kT")
        # v with per-head scaled-ones column: [seq_part, jc, head, 64 v + 1]
        v_sb = const.tile([128, JC, NHL, HD + 1], F16, tag="v")
        warm_sb = const.tile([128, 512], F16, tag="warm")

        # ---- input DMAs: phase 1 = critical set (hsT + fc0 weights +
        # consts) split across all three rings; phase 2 (wv, fc1-3) queues
        # strictly behind phase 1 on its ring.
        nc.scalar.dma_start(wq_sb[:, 0], wqP[:, 0])
        nc.scalar.dma_start(wk_sb[:, 0], wkP[:, 0])
        nc.scalar.dma_start(hsT_sb[:, 5], hsT[:, 5])
        nc.scalar.dma_start(hsT_sb[:, 7], hsT[:, 7])
        for hc in (0, 2, 4, 6):
            nc.sync.dma_start(hsT_sb[:, hc], hsT[:, hc])
        for hc in (1, 3):
            nc.gpsimd.dma_start(hsT_sb[:, hc], hsT[:, hc])
        nc.gpsimd.dma_start(bq_sb[:], bqc[:])
        nc.gpsimd.dma_start(bk_sb[:], bkc[:])
        nc.gpsimd.dma_start(mb_sb[:], mb[:])
        # phase 2
        HH = HC // 2
        nc.scalar.dma_start(wv_sb[:, 0:HH], wvT[:, 0:HH])
        nc.scalar.dma_start(wv_sb[:, HH:HC], wvT[:, HH:HC])
        nc.scalar.dma_start(wq_sb[:, 1], wqP[:, 1])
        nc.scalar.dma_start(wk_sb[:, 1], wkP[:, 1])
        for fc in (2, 3):
            nc.sync.dma_start(wq_sb[:, fc], wqP[:, fc])
            nc.sync.dma_start(wk_sb[:, fc], wkP[:, fc])

        # ---- scratch init + PE warm-up (one full HAM busy window) ----
        nc.vector.memset(warm_sb[:], 0.5)
        nc.vector.memset(v_sb[:], VSCALE)

        with tc.tile_pool(name="ps_w", bufs=1, space="PSUM") as ps_w:
            wps = ps_w.tile([128, 512], F32, tag="wu")
            for _ in range(9):
                nc.tensor.matmul(
                    wps[:], warm_sb[:, 0:128], warm_sb[:], start=True, stop=True
                )

            # ---- fc0 q/k chase: consume hsT chunks in arrival order ----
            with tc.tile_pool(name="ps_p0", bufs=1, space="PSUM") as ps_p0:
                chains = []
                for w_sb, dst, pn in ((wq_sb, qT_sb, "q"), (wk_sb, kT_sb, "k")):
                    for sc in range(SC):
                        ps = ps_p0.tile(
                            [128, 512], F32, tag=f"p0{pn}{sc}", name=f"p0{pn}{sc}"
                        )
                        chains.append((ps, w_sb, dst, sc))
                ARRIVAL = (0, 1, 2, 3, 5, 4, 7, 6)
                for step, hc in enumerate(ARRIVAL):
                    for ps, w_sb, dst, sc in chains:
                        nc.tensor.matmul(
                            ps[:],
                            w_sb[:, 0, hc],
                            hsT_sb[:, hc, sc * 512 : (sc + 1) * 512],
                            start=(step == 0),
                            stop=(step == HC - 1),
                        )
                # bias adds: q on DVE, k on ACT (parallel engines)
                for ps, w_sb, dst, sc in chains:
                    if dst is qT_sb:
                        nc.vector.tensor_scalar_add(
                            dst[:, 0, sc * 512 : (sc + 1) * 512],
                            ps[:],
                            bq_sb[:, 0:1],
                        )
                    else:
                        nc.scalar.activation(
                            dst[:, 0, sc * 512 : (sc + 1) * 512],
                            ps[:],
                            IDENT,
                            bias=bk_sb[:, 0:1],
                        )

        ps_s = ctx.enter_context(tc.tile_pool(name="ps_s", bufs=2, space="PSUM"))
        ps_c = ctx.enter_context(tc.tile_pool(name="ps_c", bufs=2, space="PSUM"))
        ps_p = ctx.enter_context(tc.tile_pool(name="ps_p", bufs=2, space="PSUM"))
        p_pool = ctx.enter_context(tc.tile_pool(name="p", bufs=5))
        sm = ctx.enter_context(tc.tile_pool(name="sm", bufs=2))

        ptbs = [None] * 8

        def emit_scores(t):
            n, jc = t // 8, t % 8
            g2, i = n >> 1, n & 1
            if jc == 0:
                ptbs[n] = p_pool.tile(
                    [128, 2, JC, 512], F16, tag="pt", name=f"ptb{n}"
                )
            ps = ps_s.tile([128, 1024], F32, tag="ss", name=f"ss{t % 2}")
            for hh in range(2):
                lo = hh * 64
                nc.tensor.matmul(
                    ps[:, hh * 512 : (hh + 1) * 512],
                    kT_sb[lo : lo + 64, g2, jc * 128 : (jc + 1) * 128],
                    qT_sb[lo : lo + 64, g2, i * 512 : (i + 1) * 512],
                    start=True,
                    stop=True,
                    tile_position=(lo, 0),
                )
            nc.scalar.activation(
                ptbs[n][:, :, jc, :],
                ps[:].rearrange("p (a b) -> p a b", a=2),
                EXP,
                bias=mb_sb[:, jc : jc + 1],
                scale=0.125,
            )

        # ---- work queue: half-chains (4 MMs each) with deadlines ----
        class Half:
            def __init__(self, fn, earliest, deadline, mms=4, prev=None):
                self.fn = fn
                self.earliest = earliest
                self.deadline = deadline
                self.mms = mms
                self.prev = prev
                self.done = False

        def make_qk_chain(fc, which, sc, earliest, deadline):
            w_sb, b_sb, dst = (
                (wq_sb, bq_sb, qT_sb) if which == 0 else (wk_sb, bk_sb, kT_sb)
            )
            st = {}

            def h1():
                st["ps"] = ps_p.tile(
                    [128, 512], F32, tag="pp", name=f"pp{fc}{which}{sc}"
                )
                for hc in range(4):
                    nc.tensor.matmul(
                        st["ps"][:],
                        w_sb[:, fc, hc],
                        hsT_sb[:, hc, sc * 512 : (sc + 1) * 512],
                        start=(hc == 0),
                        stop=False,
                    )

            def h2():
                for hc in range(4, HC):
                    nc.tensor.matmul(
                        st["ps"][:],
                        w_sb[:, fc, hc],
                        hsT_sb[:, hc, sc * 512 : (sc + 1) * 512],
                        start=False,
                        stop=(hc == HC - 1),
                    )
                nc.vector.tensor_scalar_add(
                    dst[:, fc, sc * 512 : (sc + 1) * 512],
                    st["ps"][:],
                    b_sb[:, fc : fc + 1],
                )

            a = Half(h1, earliest, deadline)
            b = Half(h2, earliest, deadline, prev=a)
            return [a, b]

        def make_v_chain(jc, earliest, deadline):
            st = {}

            def h1():
                st["ps"] = ps_p.tile([128, 512], F32, tag="pp", name=f"ppv{jc}")
                for hc in range(4):
                    nc.tensor.matmul(
                        st["ps"][:],
                        hsT_sb[:, hc, jc * 128 : (jc + 1) * 128],
                        wv_sb[:, hc, :],
                        start=(hc == 0),
                        stop=False,
                    )

            def h2():
                for hc in range(4, HC):
                    nc.tensor.matmul(
                        st["ps"][:],
                        hsT_sb[:, hc, jc * 128 : (jc + 1) * 128],
                        wv_sb[:, hc, :],
                        start=False,
                        stop=(hc == HC - 1),
                    )
                nc.vector.tensor_copy(
                    v_sb[:, jc, :, 0:HD],
                    st["ps"][:].rearrange("p (h d) -> p h d", h=NHL),
                )

            a = Half(h1, earliest, deadline)
            b = Half(h2, earliest, deadline, prev=a)
            return [a, b]

        def make_ctx_chain(n, hh):
            g2, i = n >> 1, n & 1
            h = 2 * g2 + hh
            st = {}

            def h1():
                st["ps"] = ps_c.tile(
                    [HD + 1, 512], F32, tag="cc", name=f"cc{n}{hh}"
                )
                for jc in range(4):
                    nc.tensor.matmul(
                        st["ps"][:],
                        v_sb[:, jc, h, :],
                        ptbs[n][:, hh, jc, :],
                        start=(jc == 0),
                        stop=False,
                    )

            def h2():
                for jc in range(4, JC):
                    nc.tensor.matmul(
                        st["ps"][:],
                        v_sb[:, jc, h, :],
                        ptbs[n][:, hh, jc, :],
                        start=False,
                        stop=(jc == JC - 1),
                    )
                ob = sm.tile([HD + 1, 512], F16, tag="ob", name=f"ob{n}{hh}")
                nc.vector.tensor_copy(ob[:], st["ps"][:])
                nc.sync.dma_start(out[h, :, i * 512 : (i + 1) * 512], ob[:])

            # deadline: ctx(n) must be EMITTED before scores(n+5) (step
            # 8*(n+5)-2) — the ptb pool (bufs=5) slot reuse is a WAR dep and
            # the PE queue is in-order, so late emission would deadlock.
            dl = min(63, 8 * (n + 5) - 4)
            a = Half(h1, (n + 1) * 8, dl)
            b = Half(h2, (n + 1) * 8, dl, prev=a)
            return [a, b]

        pending = []
        for fc, dl in ((1, 13), (2, 29), (3, 45)):
            for which in range(2):
                for sc in range(SC):
                    pending += make_qk_chain(fc, which, sc, 0, dl)
        for jc in range(JC):
            pending += make_v_chain(jc, 0, 22)
        for n in range(8):
            for hh in range(2):
                pending += make_ctx_chain(n, hh)

        # ---- flat schedule: 64 steps ----
        emit_scores(0)
        emit_scores(1)
        for t in range(64):
            if t + 2 < 64:
                emit_scores(t + 2)
            budget = 5
            while True:
                avail = [
                    h
                    for h in pending
                    if h.earliest <= t and (h.prev is None or h.prev not in pending)
                ]
                if not avail:
                    break
                forced = [h for h in avail if h.deadline <= t]
                if budget <= 0 and not forced:
                    break
                h = forced[0] if forced else min(avail, key=lambda x: x.deadline)
                h.fn()
                pending.remove(h)
                budget -= h.mms
        for h in pending:
            h.fn()

    nc.compile()
    return nc


_NC = None


def _get_nc():
    global _NC
    if _NC is None:
        _NC = _build_nc()
    return _NC


# test-harness knobs (ignored in normal grading use)
TRACE = False
TRACE_DIR = None
LAST_RESULT = None


def _pack(mT):
    """[1024, N] contraction-major -> [128, 8, N] partition-major fp16 so one
    DMA moves contiguous bytes per partition (big DMA packets)."""
    n = mT.shape[1]
    return np.ascontiguousarray(
        mT.reshape(HC, 128, n).transpose(1, 0, 2)
    ).astype(np.float16)


def _pack_w(w):
    """W shard [FSH, HID] -> [128, FC, HC, 128] fp16: partition = contraction
    row within hc chunk; fc-major free layout so per-fc slices are one
    contiguous 2KB run per partition."""
    return np.ascontiguousarray(
        w.T.reshape(HC, 128, FC, 128).transpose(1, 2, 0, 3)
    ).astype(np.float16)


def kernel(hidden_states, attention_mask, Wq, bq, Wk, bk, Wv, bv):
    global LAST_RESULT
    hs = np.asarray(hidden_states, dtype=np.float32)
    mask = np.asarray(attention_mask, dtype=np.float32)
    Wq = np.asarray(Wq, dtype=np.float32)
    Wk = np.asarray(Wk, dtype=np.float32)
    Wv = np.asarray(Wv, dtype=np.float32)
    bq = np.asarray(bq, dtype=np.float32)
    bk = np.asarray(bk, dtype=np.float32)
    bv = np.asarray(bv, dtype=np.float32)

    in_maps = []
    for c in range(NCORES):
        b, g = c % B, c // B
        sl = slice(g * FSH, (g + 1) * FSH)
        in_maps.append(
            {
                "hsT": _pack(hs[b].T),
                "wqP": _pack_w(Wq[sl, :]),
                "wkP": _pack_w(Wk[sl, :]),
                "wvT": _pack(Wv[sl, :].T * VSCALE),
                "bqc": np.ascontiguousarray(bq[sl].reshape(FC, 128).T),
                "bkc": np.ascontiguousarray(bk[sl].reshape(FC, 128).T),
                "mb": np.ascontiguousarray(
                    ((mask[b, 0, 0, :] - 1.0) * 1.0e6).reshape(JC, 128).T
                ),
            }
        )

    nc = _get_nc()
    kw = {}
    if TRACE:
        kw = {"trace": True, "tmpdir": TRACE_DIR}
    res = run_bass_kernel_spmd(nc, in_maps, list(range(NCORES)), **kw)
    LAST_RESULT = res

    full = np.empty((B, S, HID), dtype=np.float32)
    for c in range(NCORES):
        b, g = c % B, c // B
        o = np.asarray(res.results[c]["out"], dtype=np.float32)  # [NHL, 65, S]
        ctx = o[:, :HD, :] / o[:, HD : HD + 1, :]  # softmax divide on host
        ctx += bv[g * FSH : (g + 1) * FSH].reshape(NHL, HD, 1)
        full[b, :, g * FSH : (g + 1) * FSH] = (
            ctx.transpose(2, 0, 1).reshape(S, FSH)
        )
    return full
